# revision 9
# baseline (speedup 1.0000x reference)
"""AffinityCosineLoss on 8 Trainium2 NeuronCores.

Math: with zn = l2norm(y_pred[:, :192]), latent = (zn@zn.T + 1)/2,
target[i,j] = 0.2 (both bg) / 0.01 (one bg) / lookup[y_i,y_j] (both valid),
loss = sum_{i<j} |latent - target| / (B*(B-1)/2).

The entire pairwise computation is fused into a single K=323 matmul
P @ Q.T = latent - target by concatenating feature blocks along K:
  k 0..191   : zn_i / sqrt(2)                 (both sides)
  k 192      : 1/sqrt(2)                      (both sides; masked on Q for pads)
  k 193..320 : one-hot(y_i) on P side, -(lookup @ one-hot(y_j)) on Q side
  k 321      : b_i on P,  -0.01 - 0.18*b_j on Q     (b = is_background)
  k 322      : 1   on P,  -0.01*b_j          on Q
Then loss_sum = sum |P@Q.T| over the computed blocks.

Sharding (triangle/cyclic): the 4096x4096 pair matrix is an 8x8 grid of
512x512 super-blocks. Core r computes blocks (r, (r+d) mod 8) for d=0..4;
the d=4 slot is zero-padded on cores 4..7 (each unordered off-diagonal
block pair appears exactly once; diagonal blocks once). Host combines:
total = 2*offdiag + 1*diag - diag_elements, /2, /npairs.
"""

import functools

import ml_dtypes
import numpy as np

B = 4096
D = 256
L = 128
D_USE = 192  # int(D * 0.75)
NB = 8  # super-block grid (512 rows each)
BLK = B // NB  # 512
NSLOT = 5  # col slots per core (d = 0..4)
NCOL = NSLOT * BLK  # 2560
NT = NCOL // 128  # 20 row-tiles of the Q-side input
NSTRIP = BLK // 128  # 4 lhsT strips
N_CORES = 8
NORM_EPS = 1e-8
INV_SQRT2 = 0.7071067811865476

BF16 = ml_dtypes.bfloat16


def _build_bass():
    import concourse.bacc as bacc
    import concourse.mybir as mybir
    import concourse.tile as tile

    fp32 = mybir.dt.float32
    bf16 = mybir.dt.bfloat16
    i32 = mybir.dt.int32

    nc = bacc.Bacc("TRN2", debug=False, num_devices=N_CORES)

    ypq_d = nc.dram_tensor("ypq", [NCOL, D_USE], fp32, kind="ExternalInput")
    ylab_d = nc.dram_tensor("ylab", [1, NCOL], bf16, kind="ExternalInput")
    lkn_d = nc.dram_tensor("lkn", [L, L], bf16, kind="ExternalInput")
    wrow_d = nc.dram_tensor("wrow", [1, NCOL], bf16, kind="ExternalInput")
    qbg1_d = nc.dram_tensor("qbg1", [1, NCOL], bf16, kind="ExternalInput")
    qbg2_d = nc.dram_tensor("qbg2", [1, NCOL], bf16, kind="ExternalInput")
    pbg1_d = nc.dram_tensor("pbg1", [2, BLK], bf16, kind="ExternalInput")
    out_d = nc.dram_tensor("out", [128, NSTRIP * NSLOT], fp32, kind="ExternalOutput")

    AX = mybir.AxisListType
    ALU = mybir.AluOpType
    ACTF = mybir.ActivationFunctionType

    with tile.TileContext(nc) as tc:
        with (
            tc.tile_pool(name="cst", bufs=1) as cst,
            tc.tile_pool(name="work", bufs=1) as work,
        ):
            # ---- load inputs ----
            ylr = cst.tile([1, NCOL], bf16)
            nc.sync.dma_start(ylr[:], ylab_d.ap())
            ones_row = cst.tile([1, BLK], bf16)
            nc.gpsimd.memset(ones_row[:], 1.0)
            niota_i = cst.tile([1, 128], i32)
            nc.gpsimd.iota(niota_i[:], pattern=[[-1, 128]], base=0, channel_multiplier=0)
            niota_row = cst.tile([1, 128], bf16)
            nc.gpsimd.tensor_copy(niota_row[:], niota_i[:])
            ypf = work.tile([128, NT, D_USE], fp32)
            nc.sync.dma_start(ypf[:], ypq_d.ap().rearrange("(t p) d -> p t d", p=128))

            lkn = cst.tile([L, L], bf16)
            nc.sync.dma_start(lkn[:], lkn_d.ap())

            # ---- constants: identity (for PE transpose), iota column ----
            iota_pf = cst.tile([128, 128], i32)
            nc.gpsimd.iota(iota_pf[:], pattern=[[-1, 128]], base=0, channel_multiplier=1)
            idn = cst.tile([128, 128], bf16)
            nc.gpsimd.tensor_scalar(idn[:], iota_pf[:], 0, None, op0=ALU.is_equal)

            # ---- normalization: s = 1 / max(sqrt(2*||z||^2), sqrt(2)*eps) ----
            ypb = work.tile([128, NT, D_USE], bf16)
            nc.scalar.copy(ypb[:], ypf[:])
            sq = work.tile([128, NT, D_USE], bf16)
            nc.vector.tensor_mul(sq[:], ypb[:], ypb[:])
            norms2 = work.tile([128, NT], fp32)
            nc.vector.tensor_reduce(norms2[:], sq[:], axis=AX.X, op=ALU.add)
            rt = work.tile([128, NT], fp32)
            nc.scalar.activation(rt[:], norms2[:], ACTF.Sqrt, scale=2.0)
            rtm = work.tile([128, NT], fp32)
            nc.vector.tensor_scalar_max(rtm[:], rt[:], 1.4142135623730951 * NORM_EPS)
            s = work.tile([128, NT], fp32)
            nc.vector.reciprocal(s[:], rtm[:])

            # zn = ypb * s (per row scale), bf16
            zn = work.tile([128, NT, D_USE], bf16)
            for t in range(NT):
                nc.vector.tensor_scalar_mul(zn[:, t, :], ypb[:, t, :], s[:, t : t + 1])

            # ---- Q-side K-chunk tiles ----
            qc0 = work.tile([128, NCOL], bf16)  # zn features 0..127, transposed
            qc2 = work.tile([67, NCOL], bf16)  # feats 128..191 + const + 2 bg rows
            avt = work.tile([128, NCOL], bf16)  # one-hot labels (transposed)
            qc1 = work.tile([128, NCOL], bf16)  # -(lookup @ one-hot)

            # transpose zn into qc0 / qc2[0:64] via PE, batched 8 per PSUM bank
            with tc.tile_pool(name="pps", bufs=2, space="PSUM") as pps:
                for g in range((NT + 7) // 8):
                    ts0 = g * 8
                    ng = min(8, NT - ts0)
                    pt0 = pps.tile([128, 8 * 128], bf16, tag="pt0")
                    for i in range(ng):
                        nc.tensor.matmul(
                            pt0[:, i * 128 : (i + 1) * 128],
                            zn[:, ts0 + i, 0:128],
                            idn[:],
                            is_transpose=True,
                            start=(i == 0),
                            stop=(i == ng - 1),
                        )
                    nc.vector.tensor_copy(
                        qc0[:, ts0 * 128 : (ts0 + ng) * 128], pt0[:, 0 : ng * 128]
                    )
                    pt2 = pps.tile([64, 8 * 128], bf16, tag="pt2")
                    for i in range(ng):
                        nc.tensor.matmul(
                            pt2[:, i * 128 : (i + 1) * 128],
                            zn[:, ts0 + i, 128:D_USE],
                            idn[:],
                            is_transpose=True,
                            start=(i == 0),
                            stop=(i == ng - 1),
                        )
                    nc.scalar.copy(
                        qc2[0:64, ts0 * 128 : (ts0 + ng) * 128], pt2[:, 0 : ng * 128]
                    )

                # ---- one-hot label matrix (transposed): avt[c, j] = (y_j == c)
                # broadcast labels down partitions via PE outer product, then
                # compare against the per-partition class index on DVE.
                for j in range(NSLOT):
                    yb = pps.tile([128, BLK], fp32, tag="gv")
                    cs = slice(j * BLK, (j + 1) * BLK)
                    nc.tensor.matmul(
                        yb[:], ones_row[:, 0:128], ylr[:, cs], start=True, stop=False
                    )
                    nc.tensor.matmul(
                        yb[:], niota_row[:], ones_row[:], start=False, stop=True
                    )
                    nc.vector.tensor_scalar(
                        avt[:, cs], yb[:], 0.0, None, op0=ALU.is_equal
                    )

                # ---- qc1 = (-lookup) @ avt ----
                for j in range(NSLOT):
                    gv = pps.tile([128, BLK], fp32, tag="gv")
                    nc.tensor.matmul(
                        gv[:],
                        lkn[:],
                        avt[:, j * BLK : (j + 1) * BLK],
                        start=True,
                        stop=True,
                    )
                    if j % 2 == 0:
                        nc.vector.tensor_copy(qc1[:, j * BLK : (j + 1) * BLK], gv[:])
                    else:
                        nc.scalar.copy(qc1[:, j * BLK : (j + 1) * BLK], gv[:])

            # ---- qc2 rows 64..66 from host inputs ----
            nc.sync.dma_start(qc2[64:65, :], wrow_d.ap())
            nc.sync.dma_start(qc2[65:66, :], qbg1_d.ap())
            nc.sync.dma_start(qc2[66:67, :], qbg2_d.ap())

            # ---- P-side c2 chunk (rows 65,66 differ from Q side) ----
            pc2 = work.tile([67, BLK], bf16)
            nc.vector.tensor_copy(pc2[0:65, :], qc2[0:65, 0:BLK])
            nc.sync.dma_start(pc2[65:67, :], pbg1_d.ap())

            # ---- main: out block (strip m, slot j) = |P-block @ Q-block|, row sums
            acc = work.tile([128, NSTRIP * NSLOT], fp32)
            with tc.tile_pool(name="mps", bufs=6, space="PSUM") as mps:
                for m in range(NSTRIP):
                    c0w = qc0[:, m * 128 : (m + 1) * 128]
                    c1w = avt[:, m * 128 : (m + 1) * 128]
                    c2w = pc2[:, m * 128 : (m + 1) * 128]
                    for j in range(NSLOT):
                        u = m * NSLOT + j
                        ps = mps.tile([128, BLK], fp32, tag="mm")
                        cs = slice(j * BLK, (j + 1) * BLK)
                        nc.tensor.matmul(ps[:], c0w, qc0[:, cs], start=True, stop=False)
                        nc.tensor.matmul(ps[:], c1w, qc1[:, cs], start=False, stop=False)
                        nc.tensor.matmul(
                            ps[:], c2w, qc2[:, cs], start=False, stop=True
                        )
                        if u % 2 == 0:
                            nc.vector.tensor_reduce(
                                acc[:, u : u + 1],
                                ps[:],
                                axis=AX.X,
                                op=ALU.add,
                                apply_absolute_value=True,
                            )
                        else:
                            scr = work.tile([128, BLK], bf16, tag="scr", bufs=2)
                            nc.scalar.activation(
                                scr[:], ps[:], ACTF.Abs, accum_out=acc[:, u : u + 1]
                            )

            nc.sync.dma_start(out_d.ap(), acc[:])

    nc.compile()
    return nc


@functools.lru_cache(maxsize=1)
def _get_nc():
    return _build_bass()


def _host_inputs(y_true, y_pred, lookup):
    """Build the 8 per-core input maps."""
    yt = np.asarray(y_true).astype(np.int64)
    yp = np.asarray(y_pred).astype(np.float32)
    lk = np.asarray(lookup).astype(np.float32)

    labf = yt.astype(np.float32)  # -1 .. 127
    bg = (yt == -1).astype(np.float32)

    lkn = (-lk).astype(BF16)

    in_maps = []
    weights = np.zeros((N_CORES, NSLOT), np.float64)
    for r in range(N_CORES):
        ypq = np.zeros((NCOL, D_USE), np.float32)
        ylab = np.full((NCOL,), -2.0, np.float32)
        wrow = np.zeros((NCOL,), np.float32)
        qb1 = np.zeros((NCOL,), np.float32)
        qb2 = np.zeros((NCOL,), np.float32)
        for d in range(NSLOT):
            valid = d < 4 or r < 4
            if not valid:
                continue
            cb = (r + d) % NB
            sl = slice(d * BLK, (d + 1) * BLK)
            gsl = slice(cb * BLK, (cb + 1) * BLK)
            ypq[sl] = yp[gsl, :D_USE]
            ylab[sl] = labf[gsl]
            wrow[sl] = INV_SQRT2
            b = bg[gsl]
            qb1[sl] = -0.01 - 0.18 * b
            qb2[sl] = -0.01 * b
            weights[r, d] = 1.0 if d == 0 else 2.0
        pb1 = np.stack([bg[r * BLK : (r + 1) * BLK], np.ones(BLK, np.float32)])
        in_maps.append(
            {
                "ypq": ypq,
                "ylab": ylab.astype(BF16).reshape(1, NCOL),
                "lkn": lkn,
                "wrow": wrow.astype(BF16).reshape(1, NCOL),
                "qbg1": qb1.astype(BF16).reshape(1, NCOL),
                "qbg2": qb2.astype(BF16).reshape(1, NCOL),
                "pbg1": pb1.astype(BF16),
            }
        )
    return in_maps, weights


def _combine(outs, weights, y_true, lookup):
    """outs: list of 8 dicts with 'out' [128, NSTRIP*NSLOT]."""
    yt = np.asarray(y_true).astype(np.int64)
    lk = np.asarray(lookup).astype(np.float64)

    total = 0.0
    for r in range(N_CORES):
        o = outs[r]["out"].astype(np.float64).reshape(128, NSTRIP, NSLOT)
        per_slot = o.sum(axis=(0, 1))  # [NSLOT]
        total += float((per_slot * weights[r]).sum())

    # diagonal correction: latent_ii = 1, target_ii = 0.2 (bg) or lookup[y,y]
    bgm = yt == -1
    idx = np.clip(yt, 0, L - 1)
    tdiag = np.where(bgm, 0.2, lk[idx, idx])
    diag_sum = float(np.abs(1.0 - tdiag).sum())

    n_pairs = B * (B - 1) // 2
    return np.float32((total - diag_sum) / 2.0 / n_pairs)


def kernel(y_true, y_pred, lookup):
    from concourse.bass_utils import run_bass_kernel_spmd

    nc = _get_nc()
    in_maps, weights = _host_inputs(y_true, y_pred, lookup)
    res = run_bass_kernel_spmd(nc, in_maps, core_ids=list(range(N_CORES)))
    return _combine(res.results, weights, y_true, lookup)
